# revision 48
# baseline (speedup 1.0000x reference)
"""Confusion-matrix kernel for Trainium2 - fp8 inputs, non-uniform tiles.

Per batch b (one per core):
    pred[n]  = argmax_c input[b, c, n]
    raw[i, j] = sum_n target[b, i, n] * (pred[n] == j)
Host: cm_b = raw / rowsum(raw); out = mean_b cm_b.

y ships as fp8e4m3; x ships fp8e4m3 for tiles 2+ and fp16 for the two small
head tiles.  The ACT engine upconverts each fp8 x tile to fp16 (xf) so the
DVE max-tree and is_ge run in 2x perf mode on class-outer [P, C, Kt] tiles;
the fp16 head tiles skip conversion entirely, so the DVE starts as soon as
tile 0 lands instead of behind the conversion chain.  Tile sizes ramp up
then down ([128, 192, 384, 512, 512, 192, 128] pixels/partition): the
conversion chain stays ahead of the DVE during pipeline fill, and the small
tail tile shrinks the serial is_ge -> matmul -> store ending.

Matmul: 2-pixel packs on 128x64 column tiling (2 concurrent ldweights+matmul
lanes, lane = pack%2, PSUM partitions 64j..64j+42).  lhsT = interleaved y
pair (fp8, 42 contiguous bytes/partition, col m = 2i+a for pack pixel a,
class i), rhs = h3[:, :, 2q:2q+2] (class-outer fp16, cols n = 2c+a -> pairs
of adjacent bytes).  Each pack instruction replaces two single-pixel
matmuls: half the PE instruction stream (the 64B-per-instruction fetch from
HBM stalls the PE ~1.7us every 16KB page).  out[2i+a, 2j+b]: the a==b
diagonal sub-blocks hold the confusion matrix; host sums them across the 2
pack positions and 2 lanes, row-normalizes (absorbs fp8 argmax-tie double
counting), and means over batch.

Pipeline (2 buffers each for x8/xf/h/y):
    SP   : x loads even t; out store
    Pool : x loads odd t (first gated on x0 arrival)
    ACT  : y load issues + fp8->fp16 x conversions
    DVE  : max-tree -> is_ge(t)
    PE   : pack-matmuls(t)
"""

from contextlib import ExitStack

import ml_dtypes
import numpy as np

import concourse.bass as bass
import concourse.mybir as mybir
from concourse.bass_utils import run_bass_kernel_spmd

B, C, H, W = 8, 21, 512, 512
N = H * W
P = 128
KT = [128, 192, 384, 512, 512, 192, 128]   # pixels/partition per tile
NT = len(KT)
assert sum(KT) == N // P
KOFF = [sum(KT[:t]) for t in range(NT)]   # flat pixel offsets
KMAX = max(KT)
NH = 2                     # head tiles shipped as fp16 (no conversion)
KH = sum(KT[:NH])          # 320 head pixels/partition

N_CORES = 8

X_NP_DT = ml_dtypes.float8_e4m3
X_BIR_DT = mybir.dt.float8e4
Y_NP_DT = ml_dtypes.float8_e4m3
Y_BIR_DT = mybir.dt.float8e4

_CACHED_NC = None


def build_nc():
    nc = bass.Bass()
    # flat class-outer x / pixel-major y: per tile a contiguous [P, C*Kt] block
    x = nc.declare_dram_parameter("x", [P, C * (N // P)], X_BIR_DT, isOutput=False)
    xh = nc.declare_dram_parameter("xh", [P, C * KH], mybir.dt.float16, isOutput=False)
    y = nc.declare_dram_parameter("y", [P, C * (N // P)], Y_BIR_DT, isOutput=False)
    out = nc.declare_dram_parameter("out", [P, 2 * C], mybir.dt.float32, isOutput=True)

    with ExitStack() as ctx:
        x8 = [
            ctx.enter_context(nc.sbuf_tensor(f"x8b{i}", [P, C * KMAX], X_BIR_DT))
            for i in range(3)
        ]
        xf = [
            ctx.enter_context(nc.sbuf_tensor(f"xfb{i}", [P, C * KMAX], mybir.dt.float16))
            for i in range(2)
        ]
        ys = [
            ctx.enter_context(nc.sbuf_tensor(f"ysb{i}", [P, C * KMAX], Y_BIR_DT))
            for i in range(2)
        ]
        hs = [
            ctx.enter_context(nc.sbuf_tensor(f"hsb{i}", [P, C * KMAX], mybir.dt.float16))
            for i in range(3)
        ]
        xhs = ctx.enter_context(nc.sbuf_tensor("xhsb", [P, C * KH], mybir.dt.float16))
        ma = ctx.enter_context(nc.sbuf_tensor("ma", [P, 10 * KMAX], mybir.dt.float16))
        mb = ctx.enter_context(nc.sbuf_tensor("mb", [P, 5 * KMAX], mybir.dt.float16))
        mc = ctx.enter_context(nc.sbuf_tensor("mc", [P, 2 * KMAX], mybir.dt.float16))
        md = ctx.enter_context(nc.sbuf_tensor("md", [P, KMAX], mybir.dt.float16))
        me = ctx.enter_context(nc.sbuf_tensor("me", [P, KMAX], mybir.dt.float16))
        mm = ctx.enter_context(nc.sbuf_tensor("mm", [P, KMAX], mybir.dt.float16))
        ot = ctx.enter_context(nc.sbuf_tensor("otsb", [P, 2 * C], mybir.dt.float32))
        cm_psum = ctx.enter_context(nc.psum_tensor("cmps", [P, 2 * C], mybir.dt.float32))

        block = ctx.enter_context(nc.Block())
        sxh = [ctx.enter_context(nc.semaphore(f"sxh{i}")) for i in range(2)]
        sxs = [ctx.enter_context(nc.semaphore(f"sx{i}")) for i in range(3)]
        sys_ = [ctx.enter_context(nc.semaphore(f"sy{i}")) for i in range(2)]
        sf = ctx.enter_context(nc.semaphore("sf"))   # conversions done
        sv = ctx.enter_context(nc.semaphore("sv"))   # DVE tiles done
        sp = ctx.enter_context(nc.semaphore("sp"))   # PE tiles done
        so = ctx.enter_context(nc.semaphore("so"))

        def xin(t):
            return x[:, C * KOFF[t] : C * (KOFF[t] + KT[t])]

        def yin(t):
            return y[:, C * KOFF[t] : C * (KOFF[t] + KT[t])]

        # arrival count per buffer slot after tile t's DMA (inc 16 each)
        def arr(t):
            return 16 * (t // 2 + 1)

        def arrx(t):
            # x8 slots are cycled by tiles NH.. only
            return 16 * ((t - NH) // 3 + 1)

        def xdma(eng, t):
            # x8 slot t%3 freed once conv(t-3) consumed it (conv(t) -> sf=t-1)
            if t - 3 >= NH:
                eng.wait_ge(sf, t - 4)
            eng.dma_start(out=x8[t % 3][:, : C * KT[t]], in_=xin(t)).then_inc(
                sxs[t % 3], 16
            )

        @block.sync
        def _(sync):
            sync.dma_start(out=xhs[:], in_=xh[:]).then_inc(sxh[0], 16)
            sync.wait_ge(sxh[0], 16)      # both head tiles first and alone
            for t in range(2, NT, 2):
                xdma(sync, t)
            sync.wait_ge(sv, NT + 1)
            sync.dma_start(out=out[:], in_=ot[:]).then_inc(so, 16)
            sync.wait_ge(so, 16)

        def ydma(eng, t):
            if t >= 2:
                eng.wait_ge(sp, t - 1)    # matmuls(t-2) freed y slot
            eng.dma_start(out=ys[t % 2][:, : C * KT[t]], in_=yin(t)).then_inc(
                sys_[t % 2], 16
            )

        @block.gpsimd
        def _(gp):
            # x odd tiles + late y tiles.  Waits are ordered so no issue
            # blocks an earlier-needed one (sp/sf thresholds are increasing).
            gp.wait_ge(sxh[0], 16)        # head tiles first and alone
            xdma(gp, 3)
            ydma(gp, 2)
            xdma(gp, 5)
            for t in range(3, NT):
                ydma(gp, t)

        @block.scalar
        def _(scalar):
            # y0/y1 issues (quick), then the fp8->fp16 conversion chain
            scalar.wait_ge(sxh[0], 16)    # let x tile 0 use the engines alone
            ydma(scalar, 0)
            ydma(scalar, 1)
            for t in range(NH, NT):
                scalar.wait_ge(sxs[t % 3], arrx(t))
                if t >= NH + 2:
                    scalar.wait_ge(sv, t - 1)  # DVE(t-2) done with xf slot
                nc.scalar.activation(
                    out=xf[t % 2][:, : C * KT[t]],
                    in_=x8[t % 3][:, : C * KT[t]],
                    func=mybir.ActivationFunctionType.Copy,
                ).then_inc(sf, 1)  # sf = t - 1

        @block.vector
        def _(vector):
            TT = nc.vector.tensor_tensor
            mx = mybir.AluOpType.max
            for t in range(NT):
                k = KT[t]
                if t < NH:
                    xsrc = xhs[:, C * KOFF[t] : C * (KOFF[t] + k)]
                else:
                    xsrc = xf[t % 2][:, : C * k]
                x3 = xsrc.rearrange("p (c k) -> p c k", c=C)
                h3 = hs[t % 3][:, : C * k].rearrange("p (c k) -> p c k", c=C)
                ma3 = ma[:, : 10 * k].rearrange("p (c k) -> p c k", c=10)
                mb3 = mb[:, : 5 * k].rearrange("p (c k) -> p c k", c=5)
                mc3 = mc[:, : 2 * k].rearrange("p (c k) -> p c k", c=2)
                md3 = md[:, :k].unsqueeze(1)
                me3 = me[:, :k].unsqueeze(1)
                mm3 = mm[:, :k].unsqueeze(1)
                if t < NH:
                    vector.wait_ge(sxh[0], 16)
                else:
                    vector.wait_ge(sf, t - 1)   # conv(t) done
                TT(out=ma3, in0=x3[:, 0:10, :], in1=x3[:, 10:20, :], op=mx)
                TT(out=mb3, in0=ma3[:, 0:5, :], in1=ma3[:, 5:10, :], op=mx)
                TT(out=mc3, in0=mb3[:, 0:2, :], in1=mb3[:, 2:4, :], op=mx)
                TT(out=md3, in0=mc3[:, 0:1, :], in1=mc3[:, 1:2, :], op=mx)
                TT(out=me3, in0=md3, in1=mb3[:, 4:5, :], op=mx)
                TT(out=mm3, in0=me3, in1=x3[:, 20:21, :], op=mx)
                if t >= 3:
                    vector.wait_ge(sp, t - 2)   # matmuls(t-3) freed h slot
                TT(
                    out=h3,
                    in0=x3,
                    in1=mm3.to_broadcast((P, C, k)),
                    op=mybir.AluOpType.is_ge,
                ).then_inc(sv, 1)  # sv = t + 1
            vector.wait_ge(sp, NT)
            nc.vector.tensor_copy(ot[:], cm_psum[:]).then_inc(sv, 1)

        @block.tensor
        def _(tensor):
            for t in range(NT):
                kt = KT[t]
                yt = ys[t % 2][:]
                h3 = hs[t % 3][:, : C * kt].rearrange("p (c k) -> p c k", c=C)
                tensor.wait_ge(sv, t + 1)
                tensor.wait_ge(sys_[t % 2], arr(t))
                for q in range(kt // 2):
                    j = q % 2  # column tile lane
                    mmu = nc.tensor.matmul(
                        out=cm_psum[:][64 * j : 64 * j + 2 * C, :],
                        lhsT=yt[:, q * 2 * C : (q + 1) * 2 * C],
                        rhs=h3[:, :, 2 * q : 2 * q + 2],
                        start=(t == 0 and q == j),
                        stop=(t == NT - 1 and q == kt // 2 - 2 + j),
                        tile_position=(0, 64 * j),
                        skip_group_check=True,
                    )
                mmu.then_inc(sp, 1)

    return nc


def _get_nc():
    global _CACHED_NC
    if _CACHED_NC is None:
        _CACHED_NC = build_nc()
    return _CACHED_NC


def make_in_maps(input, target):
    inp = np.asarray(input, dtype=np.float32)
    tgt = np.asarray(target, dtype=np.float32)
    in_maps = []
    NPP = N // P   # 2048 pixels per partition
    for b in range(B):
        # class-outer per-tile blocks, concatenated: [C, NPP] per partition
        xc = inp[b].reshape(C, P, NPP).transpose(1, 0, 2)   # [P, C, NPP]
        yp = tgt[b].reshape(C, P, NPP).transpose(1, 2, 0)   # [P, NPP, C] pix-major
        xflat = np.empty((P, C * NPP), dtype=np.float32)
        for t in range(NT):
            k0, k1 = KOFF[t], KOFF[t] + KT[t]
            xflat[:, C * k0 : C * k1] = xc[:, :, k0:k1].reshape(P, C * KT[t])
        # interleave pixel pairs: pack q cols m = 2i+a = y[pixel 2q+a, class i]
        y2 = yp.reshape(P, NPP // 2, 2, C).transpose(0, 1, 3, 2)
        in_maps.append(
            {
                "x": xflat.astype(X_NP_DT),
                "xh": xflat[:, : C * KH].astype(np.float16),
                "y": np.ascontiguousarray(y2).astype(Y_NP_DT).reshape(P, C * NPP),
            }
        )
    return in_maps


def postprocess(outs):
    acc = np.stack([np.asarray(o, dtype=np.float64) for o in outs])  # [B, P, 2C]
    raw = 0
    for j in range(2):  # column tile lanes
        blk = acc[:, 64 * j : 64 * j + 2 * C, :].reshape(-1, C, 2, C, 2)
        raw = raw + blk[:, :, 0, :, 0] + blk[:, :, 1, :, 1]
    cm = raw / (raw.sum(axis=2, keepdims=True) + 1e-30)
    return cm.mean(axis=0).astype(np.float32)


def kernel(input, target):
    nc = _get_nc()
    in_maps = make_in_maps(input, target)
    res = run_bass_kernel_spmd(nc, in_maps, list(range(N_CORES)))
    return postprocess([r["out"] for r in res.results])


# revision 50
# speedup vs baseline: 1.0186x; 1.0186x over previous
"""Confusion-matrix kernel for Trainium2 - fp8 inputs, non-uniform tiles.

Per batch b (one per core):
    pred[n]  = argmax_c input[b, c, n]
    raw[i, j] = sum_n target[b, i, n] * (pred[n] == j)
Host: cm_b = raw / rowsum(raw); out = mean_b cm_b.

x and y ship as fp8e4m3 (11 MB/core total).  The ACT engine upconverts each
x tile to fp16 (xf) so the DVE max-tree and is_ge run in 2x perf mode on
class-outer [P, C, Kt] tiles.  Tile sizes ramp up then down ([128, 192, 384,
512, 512, 192, 128] pixels/partition): the ACT conversion chain stays ahead
of the DVE during pipeline fill, and the small tail tile shrinks the serial
is_ge -> matmul -> store ending.

Matmul: 2-pixel packs on 128x64 column tiling (2 concurrent ldweights+matmul
lanes, lane = pack%2, PSUM partitions 64j..64j+42).  lhsT = interleaved y
pair (fp8, 42 contiguous bytes/partition, col m = 2i+a for pack pixel a,
class i), rhs = h3[:, :, 2q:2q+2] (class-outer fp16, cols n = 2c+a -> pairs
of adjacent bytes).  Each pack instruction replaces two single-pixel
matmuls: half the PE instruction stream (the 64B-per-instruction fetch from
HBM stalls the PE ~1.7us every 16KB page).  out[2i+a, 2j+b]: the a==b
diagonal sub-blocks hold the confusion matrix; host sums them across the 2
pack positions and 2 lanes, row-normalizes (absorbs fp8 argmax-tie double
counting), and means over batch.

Pipeline (2 buffers each for x8/xf/h/y):
    SP   : x loads even t; out store
    Pool : x loads odd t (first gated on x0 arrival)
    ACT  : y load issues + fp8->fp16 x conversions
    DVE  : max-tree -> is_ge(t)
    PE   : pack-matmuls(t)
"""

from contextlib import ExitStack

import ml_dtypes
import numpy as np

import concourse.bass as bass
import concourse.mybir as mybir
from concourse.bass_utils import run_bass_kernel_spmd

B, C, H, W = 8, 21, 512, 512
N = H * W
P = 128
KT = [128, 192, 448, 512, 448, 192, 128]   # pixels/partition per tile
NT = len(KT)
assert sum(KT) == N // P
KOFF = [sum(KT[:t]) for t in range(NT)]   # flat pixel offsets
KMAX = max(KT)

N_CORES = 8

X_NP_DT = ml_dtypes.float8_e4m3
X_BIR_DT = mybir.dt.float8e4
Y_NP_DT = ml_dtypes.float8_e4m3
Y_BIR_DT = mybir.dt.float8e4

_CACHED_NC = None


def build_nc():
    nc = bass.Bass()
    # flat class-outer x / pixel-major y: per tile a contiguous [P, C*Kt] block
    x = nc.declare_dram_parameter("x", [P, C * (N // P)], X_BIR_DT, isOutput=False)
    y = nc.declare_dram_parameter("y", [P, C * (N // P)], Y_BIR_DT, isOutput=False)
    out = nc.declare_dram_parameter("out", [P, 2 * C], mybir.dt.float32, isOutput=True)

    with ExitStack() as ctx:
        x8 = [
            ctx.enter_context(nc.sbuf_tensor(f"x8b{i}", [P, C * KMAX], X_BIR_DT))
            for i in range(3)
        ]
        xf = [
            ctx.enter_context(nc.sbuf_tensor(f"xfb{i}", [P, C * KMAX], mybir.dt.float16))
            for i in range(2)
        ]
        ys = [
            ctx.enter_context(nc.sbuf_tensor(f"ysb{i}", [P, C * KMAX], Y_BIR_DT))
            for i in range(2)
        ]
        hs = [
            ctx.enter_context(nc.sbuf_tensor(f"hsb{i}", [P, C * KMAX], mybir.dt.float16))
            for i in range(3)
        ]
        ma = ctx.enter_context(nc.sbuf_tensor("ma", [P, 10 * KMAX], mybir.dt.float16))
        mb = ctx.enter_context(nc.sbuf_tensor("mb", [P, 5 * KMAX], mybir.dt.float16))
        mc = ctx.enter_context(nc.sbuf_tensor("mc", [P, 2 * KMAX], mybir.dt.float16))
        md = ctx.enter_context(nc.sbuf_tensor("md", [P, KMAX], mybir.dt.float16))
        me = ctx.enter_context(nc.sbuf_tensor("me", [P, KMAX], mybir.dt.float16))
        mm = ctx.enter_context(nc.sbuf_tensor("mm", [P, KMAX], mybir.dt.float16))
        ot = ctx.enter_context(nc.sbuf_tensor("otsb", [P, 2 * C], mybir.dt.float32))
        cm_psum = ctx.enter_context(nc.psum_tensor("cmps", [P, 2 * C], mybir.dt.float32))

        block = ctx.enter_context(nc.Block())
        sxs = [ctx.enter_context(nc.semaphore(f"sx{i}")) for i in range(3)]
        sys_ = [ctx.enter_context(nc.semaphore(f"sy{i}")) for i in range(2)]
        sf = ctx.enter_context(nc.semaphore("sf"))   # conversions done
        sv = ctx.enter_context(nc.semaphore("sv"))   # DVE tiles done
        sp = ctx.enter_context(nc.semaphore("sp"))   # PE tiles done
        so = ctx.enter_context(nc.semaphore("so"))

        def xin(t):
            return x[:, C * KOFF[t] : C * (KOFF[t] + KT[t])]

        def yin(t):
            return y[:, C * KOFF[t] : C * (KOFF[t] + KT[t])]

        # arrival count per buffer slot after tile t's DMA (inc 16 each)
        def arr(t):
            return 16 * (t // 2 + 1)

        def arr3(t):
            return 16 * (t // 3 + 1)

        def xdma(eng, t):
            # x8 slot t%3 freed once conv(t-3) consumed it
            if t >= 3:
                eng.wait_ge(sf, t - 2)
            eng.dma_start(out=x8[t % 3][:, : C * KT[t]], in_=xin(t)).then_inc(
                sxs[t % 3], 16
            )

        @block.sync
        def _(sync):
            xdma(sync, 0)
            sync.wait_ge(sxs[0], 16)      # x0 first and alone
            for t in range(2, NT, 2):
                xdma(sync, t)
            sync.wait_ge(sv, NT + 1)
            sync.dma_start(out=out[:], in_=ot[:]).then_inc(so, 16)
            sync.wait_ge(so, 16)

        def ydma(eng, t):
            if t >= 2:
                eng.wait_ge(sp, t - 1)    # matmuls(t-2) freed y slot
            eng.dma_start(out=ys[t % 2][:, : C * KT[t]], in_=yin(t)).then_inc(
                sys_[t % 2], 16
            )

        @block.gpsimd
        def _(gp):
            # x odd tiles + late y tiles.  Waits are ordered so no issue
            # blocks an earlier-needed one (sp/sf thresholds are increasing).
            gp.wait_ge(sxs[0], 16)        # x0 first and alone
            xdma(gp, 1)
            xdma(gp, 3)
            ydma(gp, 2)
            xdma(gp, 5)
            for t in range(3, NT):
                ydma(gp, t)

        @block.scalar
        def _(scalar):
            # y0/y1 issues (quick), then the fp8->fp16 conversion chain
            scalar.wait_ge(sxs[0], 16)    # let x0 use the DMA engines alone
            ydma(scalar, 0)
            ydma(scalar, 1)
            for t in range(NT):
                scalar.wait_ge(sxs[t % 3], arr3(t))
                if t >= 2:
                    scalar.wait_ge(sv, t - 1)  # DVE(t-2) done with xf slot
                nc.scalar.activation(
                    out=xf[t % 2][:, : C * KT[t]],
                    in_=x8[t % 3][:, : C * KT[t]],
                    func=mybir.ActivationFunctionType.Copy,
                ).then_inc(sf, 1)  # sf = t + 1

        @block.vector
        def _(vector):
            TT = nc.vector.tensor_tensor
            mx = mybir.AluOpType.max
            for t in range(NT):
                k = KT[t]
                x3 = xf[t % 2][:, : C * k].rearrange("p (c k) -> p c k", c=C)
                h3 = hs[t % 3][:, : C * k].rearrange("p (c k) -> p c k", c=C)
                ma3 = ma[:, : 10 * k].rearrange("p (c k) -> p c k", c=10)
                mb3 = mb[:, : 5 * k].rearrange("p (c k) -> p c k", c=5)
                mc3 = mc[:, : 2 * k].rearrange("p (c k) -> p c k", c=2)
                md3 = md[:, :k].unsqueeze(1)
                me3 = me[:, :k].unsqueeze(1)
                mm3 = mm[:, :k].unsqueeze(1)
                vector.wait_ge(sf, t + 1)
                TT(out=ma3, in0=x3[:, 0:10, :], in1=x3[:, 10:20, :], op=mx)
                TT(out=mb3, in0=ma3[:, 0:5, :], in1=ma3[:, 5:10, :], op=mx)
                TT(out=mc3, in0=mb3[:, 0:2, :], in1=mb3[:, 2:4, :], op=mx)
                TT(out=md3, in0=mc3[:, 0:1, :], in1=mc3[:, 1:2, :], op=mx)
                TT(out=me3, in0=md3, in1=mb3[:, 4:5, :], op=mx)
                TT(out=mm3, in0=me3, in1=x3[:, 20:21, :], op=mx)
                if t >= 3:
                    vector.wait_ge(sp, t - 2)   # matmuls(t-3) freed h slot
                TT(
                    out=h3,
                    in0=x3,
                    in1=mm3.to_broadcast((P, C, k)),
                    op=mybir.AluOpType.is_ge,
                ).then_inc(sv, 1)  # sv = t + 1
            vector.wait_ge(sp, NT)
            nc.vector.tensor_copy(ot[:], cm_psum[:]).then_inc(sv, 1)

        @block.tensor
        def _(tensor):
            for t in range(NT):
                kt = KT[t]
                yt = ys[t % 2][:]
                h3 = hs[t % 3][:, : C * kt].rearrange("p (c k) -> p c k", c=C)
                tensor.wait_ge(sv, t + 1)
                tensor.wait_ge(sys_[t % 2], arr(t))
                for q in range(kt // 2):
                    j = q % 2  # column tile lane
                    mmu = nc.tensor.matmul(
                        out=cm_psum[:][64 * j : 64 * j + 2 * C, :],
                        lhsT=yt[:, q * 2 * C : (q + 1) * 2 * C],
                        rhs=h3[:, :, 2 * q : 2 * q + 2],
                        start=(t == 0 and q == j),
                        stop=(t == NT - 1 and q == kt // 2 - 2 + j),
                        tile_position=(0, 64 * j),
                        skip_group_check=True,
                    )
                mmu.then_inc(sp, 1)

    return nc


def _get_nc():
    global _CACHED_NC
    if _CACHED_NC is None:
        _CACHED_NC = build_nc()
    return _CACHED_NC


def make_in_maps(input, target):
    inp = np.asarray(input, dtype=np.float32)
    tgt = np.asarray(target, dtype=np.float32)
    in_maps = []
    NPP = N // P   # 2048 pixels per partition
    for b in range(B):
        # class-outer per-tile blocks, concatenated: [C, NPP] per partition
        xc = inp[b].reshape(C, P, NPP).transpose(1, 0, 2)   # [P, C, NPP]
        yp = tgt[b].reshape(C, P, NPP).transpose(1, 2, 0)   # [P, NPP, C] pix-major
        xflat = np.empty((P, C * NPP), dtype=np.float32)
        for t in range(NT):
            k0, k1 = KOFF[t], KOFF[t] + KT[t]
            xflat[:, C * k0 : C * k1] = xc[:, :, k0:k1].reshape(P, C * KT[t])
        # interleave pixel pairs: pack q cols m = 2i+a = y[pixel 2q+a, class i]
        y2 = yp.reshape(P, NPP // 2, 2, C).transpose(0, 1, 3, 2)
        in_maps.append(
            {
                "x": xflat.astype(X_NP_DT),
                "y": np.ascontiguousarray(y2).astype(Y_NP_DT).reshape(P, C * NPP),
            }
        )
    return in_maps


def postprocess(outs):
    acc = np.stack([np.asarray(o, dtype=np.float64) for o in outs])  # [B, P, 2C]
    raw = 0
    for j in range(2):  # column tile lanes
        blk = acc[:, 64 * j : 64 * j + 2 * C, :].reshape(-1, C, 2, C, 2)
        raw = raw + blk[:, :, 0, :, 0] + blk[:, :, 1, :, 1]
    cm = raw / (raw.sum(axis=2, keepdims=True) + 1e-30)
    return cm.mean(axis=0).astype(np.float32)


def kernel(input, target):
    nc = _get_nc()
    in_maps = make_in_maps(input, target)
    res = run_bass_kernel_spmd(nc, in_maps, list(range(N_CORES)))
    return postprocess([r["out"] for r in res.results])


# revision 54
# speedup vs baseline: 1.0696x; 1.0502x over previous
"""Confusion-matrix kernel for Trainium2 - fp8 inputs, non-uniform tiles.

Per batch b (one per core):
    pred[n]  = argmax_c input[b, c, n]
    raw[i, j] = sum_n target[b, i, n] * (pred[n] == j)
Host: cm_b = raw / rowsum(raw); out = mean_b cm_b.

x and y ship as fp8e4m3 (11 MB/core total).  The ACT engine upconverts each
x tile to fp16 (xf) so the DVE max-tree and is_ge run in 2x perf mode on
class-outer [P, C, Kt] tiles.  Tile sizes ramp up then down ([128, 192, 384,
512, 512, 192, 128] pixels/partition): the ACT conversion chain stays ahead
of the DVE during pipeline fill, and the small tail tile shrinks the serial
is_ge -> matmul -> store ending.

Matmul: 2-pixel packs on 128x64 column tiling (2 concurrent ldweights+matmul
lanes, lane = pack%2, PSUM partitions 64j..64j+42).  lhsT = interleaved y
pair (fp8, 42 contiguous bytes/partition, col m = 2i+a for pack pixel a,
class i), rhs = h3[:, :, 2q:2q+2] (class-outer fp16, cols n = 2c+a -> pairs
of adjacent bytes).  Each pack instruction replaces two single-pixel
matmuls: half the PE instruction stream (the 64B-per-instruction fetch from
HBM stalls the PE ~1.7us every 16KB page).  out[2i+a, 2j+b]: the a==b
diagonal sub-blocks hold the confusion matrix; host sums them across the 2
pack positions and 2 lanes, row-normalizes (absorbs fp8 argmax-tie double
counting), and means over batch.

Pipeline (2 buffers each for x8/xf/h/y):
    SP   : x loads even t; out store
    Pool : x loads odd t (first gated on x0 arrival)
    ACT  : y load issues + fp8->fp16 x conversions
    DVE  : max-tree -> is_ge(t)
    PE   : pack-matmuls(t)
"""

from contextlib import ExitStack

import ml_dtypes
import numpy as np

import concourse.bass as bass
import concourse.mybir as mybir
from concourse.bass_utils import run_bass_kernel_spmd

B, C, H, W = 8, 21, 512, 512
N = H * W
P = 128
KT = [128, 192, 384, 512, 512, 192, 128]   # pixels/partition per tile
NT = len(KT)
assert sum(KT) == N // P
KOFF = [sum(KT[:t]) for t in range(NT)]   # flat pixel offsets
KMAX = max(KT)

N_CORES = 8

X_NP_DT = ml_dtypes.float8_e4m3
X_BIR_DT = mybir.dt.float8e4
Y_NP_DT = ml_dtypes.float8_e4m3
Y_BIR_DT = mybir.dt.float8e4

_CACHED_NC = None


def build_nc():
    nc = bass.Bass()
    # flat class-outer x / pixel-major y: per tile a contiguous [P, C*Kt] block
    x = nc.declare_dram_parameter("x", [P, C * (N // P)], X_BIR_DT, isOutput=False)
    y = nc.declare_dram_parameter("y", [P, C * (N // P)], Y_BIR_DT, isOutput=False)
    out = nc.declare_dram_parameter("out", [P, 2 * C], mybir.dt.float32, isOutput=True)

    with ExitStack() as ctx:
        x8 = [
            ctx.enter_context(nc.sbuf_tensor(f"x8b{i}", [P, C * KMAX], X_BIR_DT))
            for i in range(3)
        ]
        xf = [
            ctx.enter_context(nc.sbuf_tensor(f"xfb{i}", [P, C * KMAX], mybir.dt.float16))
            for i in range(2)
        ]
        ys = [
            ctx.enter_context(nc.sbuf_tensor(f"ysb{i}", [P, C * KMAX], Y_BIR_DT))
            for i in range(2)
        ]
        hs = [
            ctx.enter_context(nc.sbuf_tensor(f"hsb{i}", [P, C * KMAX], mybir.dt.float16))
            for i in range(3)
        ]
        ma = ctx.enter_context(nc.sbuf_tensor("ma", [P, 10 * KMAX], mybir.dt.float16))
        mb = ctx.enter_context(nc.sbuf_tensor("mb", [P, 5 * KMAX], mybir.dt.float16))
        mc = ctx.enter_context(nc.sbuf_tensor("mc", [P, 2 * KMAX], mybir.dt.float16))
        md = ctx.enter_context(nc.sbuf_tensor("md", [P, KMAX], mybir.dt.float16))
        me = ctx.enter_context(nc.sbuf_tensor("me", [P, KMAX], mybir.dt.float16))
        mm = ctx.enter_context(nc.sbuf_tensor("mm", [P, KMAX], mybir.dt.float16))
        ot = ctx.enter_context(nc.sbuf_tensor("otsb", [P, 2 * C], mybir.dt.float32))
        cm_psum = ctx.enter_context(nc.psum_tensor("cmps", [P, 2 * C], mybir.dt.float32))

        block = ctx.enter_context(nc.Block())
        sxs = [ctx.enter_context(nc.semaphore(f"sx{i}")) for i in range(3)]
        sys_ = [ctx.enter_context(nc.semaphore(f"sy{i}")) for i in range(2)]
        sf = ctx.enter_context(nc.semaphore("sf"))   # conversions done
        svh = ctx.enter_context(nc.semaphore("svh"))  # is_ge first halves done
        sv = ctx.enter_context(nc.semaphore("sv"))   # DVE tiles done
        sp = ctx.enter_context(nc.semaphore("sp"))   # PE tiles done
        so = ctx.enter_context(nc.semaphore("so"))

        def xin(t):
            return x[:, C * KOFF[t] : C * (KOFF[t] + KT[t])]

        def yin(t):
            return y[:, C * KOFF[t] : C * (KOFF[t] + KT[t])]

        # arrival count per buffer slot after tile t's DMA (inc 16 each)
        def arr(t):
            return 16 * (t // 2 + 1)

        def arr3(t):
            return 16 * (t // 3 + 1)

        def xdma(eng, t):
            # x8 slot t%3 freed once conv(t-3) consumed it
            if t >= 3:
                eng.wait_ge(sf, t - 2)
            eng.dma_start(out=x8[t % 3][:, : C * KT[t]], in_=xin(t)).then_inc(
                sxs[t % 3], 16
            )

        @block.sync
        def _(sync):
            xdma(sync, 0)
            sync.wait_ge(sxs[0], 16)      # x0 first and alone
            for t in range(2, NT, 2):
                xdma(sync, t)
            sync.wait_ge(sv, NT + 1)
            sync.dma_start(out=out[:], in_=ot[:]).then_inc(so, 16)
            sync.wait_ge(so, 16)

        def ydma(eng, t):
            if t >= 2:
                eng.wait_ge(sp, t - 1)    # matmuls(t-2) freed y slot
            eng.dma_start(out=ys[t % 2][:, : C * KT[t]], in_=yin(t)).then_inc(
                sys_[t % 2], 16
            )

        @block.gpsimd
        def _(gp):
            # x odd tiles + late y tiles.  Waits are ordered so no issue
            # blocks an earlier-needed one (sp/sf thresholds are increasing).
            gp.wait_ge(sxs[0], 16)        # x0 first and alone
            xdma(gp, 1)
            xdma(gp, 3)
            ydma(gp, 2)
            xdma(gp, 5)
            for t in range(3, NT):
                ydma(gp, t)

        @block.scalar
        def _(scalar):
            # y0/y1 issues (quick), then the fp8->fp16 conversion chain
            scalar.wait_ge(sxs[0], 16)    # let x0 use the DMA engines alone
            ydma(scalar, 0)
            ydma(scalar, 1)
            for t in range(NT):
                scalar.wait_ge(sxs[t % 3], arr3(t))
                if t >= 2:
                    scalar.wait_ge(sv, t - 1)  # DVE(t-2) done with xf slot
                nc.scalar.activation(
                    out=xf[t % 2][:, : C * KT[t]],
                    in_=x8[t % 3][:, : C * KT[t]],
                    func=mybir.ActivationFunctionType.Copy,
                ).then_inc(sf, 1)  # sf = t + 1

        @block.vector
        def _(vector):
            TT = nc.vector.tensor_tensor
            mx = mybir.AluOpType.max
            for t in range(NT):
                k = KT[t]
                x3 = xf[t % 2][:, : C * k].rearrange("p (c k) -> p c k", c=C)
                h3 = hs[t % 3][:, : C * k].rearrange("p (c k) -> p c k", c=C)
                ma3 = ma[:, : 10 * k].rearrange("p (c k) -> p c k", c=10)
                mb3 = mb[:, : 5 * k].rearrange("p (c k) -> p c k", c=5)
                mc3 = mc[:, : 2 * k].rearrange("p (c k) -> p c k", c=2)
                md3 = md[:, :k].unsqueeze(1)
                me3 = me[:, :k].unsqueeze(1)
                mm3 = mm[:, :k].unsqueeze(1)
                vector.wait_ge(sf, t + 1)
                TT(out=ma3, in0=x3[:, 0:10, :], in1=x3[:, 10:20, :], op=mx)
                TT(out=mb3, in0=ma3[:, 0:5, :], in1=ma3[:, 5:10, :], op=mx)
                TT(out=mc3, in0=mb3[:, 0:2, :], in1=mb3[:, 2:4, :], op=mx)
                TT(out=md3, in0=mc3[:, 0:1, :], in1=mc3[:, 1:2, :], op=mx)
                TT(out=me3, in0=md3, in1=mb3[:, 4:5, :], op=mx)
                TT(out=mm3, in0=me3, in1=x3[:, 20:21, :], op=mx)
                if t >= 3:
                    vector.wait_ge(sp, t - 2)   # matmuls(t-3) freed h slot
                if t in (3, 4):
                    # split is_ge so PE starts (and fetches its instruction
                    # pages) on the first half-tile early
                    h = k // 2
                    TT(
                        out=h3[:, :, 0:h],
                        in0=x3[:, :, 0:h],
                        in1=mm[:, 0:h].unsqueeze(1).to_broadcast((P, C, h)),
                        op=mybir.AluOpType.is_ge,
                    ).then_inc(svh, 1)  # svh = t - 2
                    TT(
                        out=h3[:, :, h:k],
                        in0=x3[:, :, h:k],
                        in1=mm[:, h:k].unsqueeze(1).to_broadcast((P, C, h)),
                        op=mybir.AluOpType.is_ge,
                    ).then_inc(sv, 1)  # sv = t + 1
                else:
                    TT(
                        out=h3,
                        in0=x3,
                        in1=mm3.to_broadcast((P, C, k)),
                        op=mybir.AluOpType.is_ge,
                    ).then_inc(sv, 1)  # sv = t + 1
            vector.wait_ge(sp, NT)
            nc.vector.tensor_copy(ot[:], cm_psum[:]).then_inc(sv, 1)

        @block.tensor
        def _(tensor):
            for t in range(NT):
                kt = KT[t]
                yt = ys[t % 2][:]
                h3 = hs[t % 3][:, : C * kt].rearrange("p (c k) -> p c k", c=C)
                tensor.wait_ge(sys_[t % 2], arr(t))
                if t in (3, 4):
                    tensor.wait_ge(svh, t - 2)  # is_ge(t) first half done
                else:
                    tensor.wait_ge(sv, t + 1)
                for q in range(kt // 2):
                    if t in (3, 4) and q == kt // 4:
                        tensor.wait_ge(sv, t + 1)  # second half done
                    j = q % 2  # column tile lane
                    mmu = nc.tensor.matmul(
                        out=cm_psum[:][64 * j : 64 * j + 2 * C, :],
                        lhsT=yt[:, q * 2 * C : (q + 1) * 2 * C],
                        rhs=h3[:, :, 2 * q : 2 * q + 2],
                        start=(t == 0 and q == j),
                        stop=(t == NT - 1 and q == kt // 2 - 2 + j),
                        tile_position=(0, 64 * j),
                        skip_group_check=True,
                    )
                mmu.then_inc(sp, 1)

    return nc


def _get_nc():
    global _CACHED_NC
    if _CACHED_NC is None:
        _CACHED_NC = build_nc()
    return _CACHED_NC


def make_in_maps(input, target):
    inp = np.asarray(input, dtype=np.float32)
    tgt = np.asarray(target, dtype=np.float32)
    in_maps = []
    NPP = N // P   # 2048 pixels per partition
    for b in range(B):
        # class-outer per-tile blocks, concatenated: [C, NPP] per partition
        xc = inp[b].reshape(C, P, NPP).transpose(1, 0, 2)   # [P, C, NPP]
        yp = tgt[b].reshape(C, P, NPP).transpose(1, 2, 0)   # [P, NPP, C] pix-major
        xflat = np.empty((P, C * NPP), dtype=np.float32)
        for t in range(NT):
            k0, k1 = KOFF[t], KOFF[t] + KT[t]
            xflat[:, C * k0 : C * k1] = xc[:, :, k0:k1].reshape(P, C * KT[t])
        # interleave pixel pairs: pack q cols m = 2i+a = y[pixel 2q+a, class i]
        y2 = yp.reshape(P, NPP // 2, 2, C).transpose(0, 1, 3, 2)
        in_maps.append(
            {
                "x": xflat.astype(X_NP_DT),
                "y": np.ascontiguousarray(y2).astype(Y_NP_DT).reshape(P, C * NPP),
            }
        )
    return in_maps


def postprocess(outs):
    acc = np.stack([np.asarray(o, dtype=np.float64) for o in outs])  # [B, P, 2C]
    raw = 0
    for j in range(2):  # column tile lanes
        blk = acc[:, 64 * j : 64 * j + 2 * C, :].reshape(-1, C, 2, C, 2)
        raw = raw + blk[:, :, 0, :, 0] + blk[:, :, 1, :, 1]
    cm = raw / (raw.sum(axis=2, keepdims=True) + 1e-30)
    return cm.mean(axis=0).astype(np.float32)


def kernel(input, target):
    nc = _get_nc()
    in_maps = make_in_maps(input, target)
    res = run_bass_kernel_spmd(nc, in_maps, list(range(N_CORES)))
    return postprocess([r["out"] for r in res.results])


# revision 57
# speedup vs baseline: 1.0810x; 1.0106x over previous
"""Confusion-matrix kernel for Trainium2 - fp8 inputs, non-uniform tiles.

Per batch b (one per core):
    pred[n]  = argmax_c input[b, c, n]
    raw[i, j] = sum_n target[b, i, n] * (pred[n] == j)
Host: cm_b = raw / rowsum(raw); out = mean_b cm_b.

x and y ship as fp8e4m3 (11 MB/core total).  The ACT engine upconverts each
x tile to fp16 (xf) so the DVE max-tree and is_ge run in 2x perf mode on
class-outer [P, C, Kt] tiles.  Tile sizes ramp up then down ([128, 192, 384,
512, 512, 192, 128] pixels/partition): the ACT conversion chain stays ahead
of the DVE during pipeline fill, and the small tail tile shrinks the serial
is_ge -> matmul -> store ending.

Matmul: 2-pixel packs on 128x64 column tiling (2 concurrent ldweights+matmul
lanes, lane = pack%2, PSUM partitions 64j..64j+42).  lhsT = interleaved y
pair (fp8, 42 contiguous bytes/partition, col m = 2i+a for pack pixel a,
class i), rhs = h3[:, :, 2q:2q+2] (class-outer fp16, cols n = 2c+a -> pairs
of adjacent bytes).  Each pack instruction replaces two single-pixel
matmuls: half the PE instruction stream (the 64B-per-instruction fetch from
HBM stalls the PE ~1.7us every 16KB page).  out[2i+a, 2j+b]: the a==b
diagonal sub-blocks hold the confusion matrix; host sums them across the 2
pack positions and 2 lanes, row-normalizes (absorbs fp8 argmax-tie double
counting), and means over batch.

Pipeline (2 buffers each for x8/xf/h/y):
    SP   : x loads even t; out store
    Pool : x loads odd t (first gated on x0 arrival)
    ACT  : y load issues + fp8->fp16 x conversions
    DVE  : max-tree -> is_ge(t)
    PE   : pack-matmuls(t)
"""

from contextlib import ExitStack

import ml_dtypes
import numpy as np

import concourse.bass as bass
import concourse.mybir as mybir
from concourse.bass_utils import run_bass_kernel_spmd

B, C, H, W = 8, 21, 512, 512
N = H * W
P = 128
KT = [128, 192, 384, 512, 512, 192, 128]   # pixels/partition per tile
NT = len(KT)
assert sum(KT) == N // P
KOFF = [sum(KT[:t]) for t in range(NT)]   # flat pixel offsets
KMAX = max(KT)

N_CORES = 8

X_NP_DT = ml_dtypes.float8_e4m3
X_BIR_DT = mybir.dt.float8e4
Y_NP_DT = ml_dtypes.float8_e4m3
Y_BIR_DT = mybir.dt.float8e4

_CACHED_NC = None


def build_nc():
    nc = bass.Bass()
    # flat class-outer x / pixel-major y: per tile a contiguous [P, C*Kt] block
    x = nc.declare_dram_parameter("x", [P, C * (N // P)], X_BIR_DT, isOutput=False)
    y = nc.declare_dram_parameter("y", [P, C * (N // P)], Y_BIR_DT, isOutput=False)
    out = nc.declare_dram_parameter("out", [P, 2 * C], mybir.dt.float32, isOutput=True)

    with ExitStack() as ctx:
        x8 = [
            ctx.enter_context(nc.sbuf_tensor(f"x8b{i}", [P, C * KMAX], X_BIR_DT))
            for i in range(3)
        ]
        xf = [
            ctx.enter_context(nc.sbuf_tensor(f"xfb{i}", [P, C * KMAX], mybir.dt.float16))
            for i in range(2)
        ]
        ys = [
            ctx.enter_context(nc.sbuf_tensor(f"ysb{i}", [P, C * KMAX], Y_BIR_DT))
            for i in range(2)
        ]
        hs = [
            ctx.enter_context(nc.sbuf_tensor(f"hsb{i}", [P, C * KMAX], mybir.dt.float16))
            for i in range(3)
        ]
        ma = ctx.enter_context(nc.sbuf_tensor("ma", [P, 10 * KMAX], mybir.dt.float16))
        mb = ctx.enter_context(nc.sbuf_tensor("mb", [P, 5 * KMAX], mybir.dt.float16))
        mc = ctx.enter_context(nc.sbuf_tensor("mc", [P, 2 * KMAX], mybir.dt.float16))
        md = ctx.enter_context(nc.sbuf_tensor("md", [P, KMAX], mybir.dt.float16))
        me = ctx.enter_context(nc.sbuf_tensor("me", [P, KMAX], mybir.dt.float16))
        mm = ctx.enter_context(nc.sbuf_tensor("mm", [P, KMAX], mybir.dt.float16))
        ot = ctx.enter_context(nc.sbuf_tensor("otsb", [P, 2 * C], mybir.dt.float32))
        cm_psum = ctx.enter_context(nc.psum_tensor("cmps", [P, 2 * C], mybir.dt.float32))

        block = ctx.enter_context(nc.Block())
        sxs = [ctx.enter_context(nc.semaphore(f"sx{i}")) for i in range(3)]
        sys_ = [ctx.enter_context(nc.semaphore(f"sy{i}")) for i in range(2)]
        sf = ctx.enter_context(nc.semaphore("sf"))   # conversions done
        svh = ctx.enter_context(nc.semaphore("svh"))  # is_ge first halves done
        sv = ctx.enter_context(nc.semaphore("sv"))   # DVE tiles done
        sp = ctx.enter_context(nc.semaphore("sp"))   # PE tiles done
        so = ctx.enter_context(nc.semaphore("so"))

        def xin(t):
            return x[:, C * KOFF[t] : C * (KOFF[t] + KT[t])]

        def yin(t):
            return y[:, C * KOFF[t] : C * (KOFF[t] + KT[t])]

        # arrival count per buffer slot after tile t's DMA (inc 16 each)
        def arr(t):
            return 16 * (t // 2 + 1)

        def arr3(t):
            return 16 * (t // 3 + 1)

        def xdma(eng, t):
            # x8 slot t%3 freed once conv(t-3) consumed it
            if t >= 3:
                eng.wait_ge(sf, t - 2)
            eng.dma_start(out=x8[t % 3][:, : C * KT[t]], in_=xin(t)).then_inc(
                sxs[t % 3], 16
            )

        @block.sync
        def _(sync):
            xdma(sync, 0)
            sync.wait_ge(sxs[0], 16)      # x0 first and alone
            for t in range(2, NT, 2):
                xdma(sync, t)
            sync.wait_ge(sv, NT + 1)
            sync.dma_start(out=out[:], in_=ot[:]).then_inc(so, 16)
            sync.wait_ge(so, 16)

        def ydma(eng, t):
            if t >= 2:
                eng.wait_ge(sp, t - 1)    # matmuls(t-2) freed y slot
            eng.dma_start(out=ys[t % 2][:, : C * KT[t]], in_=yin(t)).then_inc(
                sys_[t % 2], 16
            )

        @block.gpsimd
        def _(gp):
            # x odd tiles + late y tiles.  Waits are ordered so no issue
            # blocks an earlier-needed one (sp/sf thresholds are increasing).
            gp.wait_ge(sxs[0], 16)        # x0 first and alone
            xdma(gp, 1)
            xdma(gp, 3)
            ydma(gp, 2)
            xdma(gp, 5)
            for t in range(3, NT):
                ydma(gp, t)

        @block.scalar
        def _(scalar):
            # y0/y1 issues (quick), then the fp8->fp16 conversion chain
            scalar.wait_ge(sxs[0], 16)    # let x0 use the DMA engines alone
            ydma(scalar, 0)
            ydma(scalar, 1)
            for t in range(NT):
                scalar.wait_ge(sxs[t % 3], arr3(t))
                if t >= 2:
                    scalar.wait_ge(sv, t - 1)  # DVE(t-2) done with xf slot
                nc.scalar.activation(
                    out=xf[t % 2][:, : C * KT[t]],
                    in_=x8[t % 3][:, : C * KT[t]],
                    func=mybir.ActivationFunctionType.Copy,
                ).then_inc(sf, 1)  # sf = t + 1

        @block.vector
        def _(vector):
            TT = nc.vector.tensor_tensor
            mx = mybir.AluOpType.max
            for t in range(NT):
                k = KT[t]
                x3 = xf[t % 2][:, : C * k].rearrange("p (c k) -> p c k", c=C)
                h3 = hs[t % 3][:, : C * k].rearrange("p (c k) -> p c k", c=C)
                ma3 = ma[:, : 10 * k].rearrange("p (c k) -> p c k", c=10)
                mb3 = mb[:, : 5 * k].rearrange("p (c k) -> p c k", c=5)
                mc3 = mc[:, : 2 * k].rearrange("p (c k) -> p c k", c=2)
                md3 = md[:, :k].unsqueeze(1)
                me3 = me[:, :k].unsqueeze(1)
                mm3 = mm[:, :k].unsqueeze(1)
                vector.wait_ge(sf, t + 1)
                TT(out=ma3, in0=x3[:, 0:10, :], in1=x3[:, 10:20, :], op=mx)
                TT(out=mb3, in0=ma3[:, 0:5, :], in1=ma3[:, 5:10, :], op=mx)
                TT(out=mc3, in0=mb3[:, 0:2, :], in1=mb3[:, 2:4, :], op=mx)
                TT(out=md3, in0=mc3[:, 0:1, :], in1=mc3[:, 1:2, :], op=mx)
                TT(out=me3, in0=md3, in1=mb3[:, 4:5, :], op=mx)
                TT(out=mm3, in0=me3, in1=x3[:, 20:21, :], op=mx)
                if t >= 3:
                    vector.wait_ge(sp, t - 2)   # matmuls(t-3) freed h slot
                if t in (2, 3, 4, 5):
                    # split is_ge so PE starts (and fetches its instruction
                    # pages) on the first half-tile early
                    h = k // 2
                    TT(
                        out=h3[:, :, 0:h],
                        in0=x3[:, :, 0:h],
                        in1=mm[:, 0:h].unsqueeze(1).to_broadcast((P, C, h)),
                        op=mybir.AluOpType.is_ge,
                    ).then_inc(svh, 1)  # svh = t - 1
                    TT(
                        out=h3[:, :, h:k],
                        in0=x3[:, :, h:k],
                        in1=mm[:, h:k].unsqueeze(1).to_broadcast((P, C, h)),
                        op=mybir.AluOpType.is_ge,
                    ).then_inc(sv, 1)  # sv = t + 1
                else:
                    TT(
                        out=h3,
                        in0=x3,
                        in1=mm3.to_broadcast((P, C, k)),
                        op=mybir.AluOpType.is_ge,
                    ).then_inc(sv, 1)  # sv = t + 1
            vector.wait_ge(sp, NT)
            nc.vector.tensor_copy(ot[:], cm_psum[:]).then_inc(sv, 1)

        @block.tensor
        def _(tensor):
            for t in range(NT):
                kt = KT[t]
                yt = ys[t % 2][:]
                h3 = hs[t % 3][:, : C * kt].rearrange("p (c k) -> p c k", c=C)
                tensor.wait_ge(sys_[t % 2], arr(t))
                if t in (2, 3, 4, 5):
                    tensor.wait_ge(svh, t - 1)  # is_ge(t) first half done
                else:
                    tensor.wait_ge(sv, t + 1)
                for q in range(kt // 2):
                    if t in (2, 3, 4, 5) and q == kt // 4:
                        tensor.wait_ge(sv, t + 1)  # second half done
                    j = q % 2  # column tile lane
                    mmu = nc.tensor.matmul(
                        out=cm_psum[:][64 * j : 64 * j + 2 * C, :],
                        lhsT=yt[:, q * 2 * C : (q + 1) * 2 * C],
                        rhs=h3[:, :, 2 * q : 2 * q + 2],
                        start=(t == 0 and q == j),
                        stop=(t == NT - 1 and q == kt // 2 - 2 + j),
                        tile_position=(0, 64 * j),
                        skip_group_check=True,
                    )
                mmu.then_inc(sp, 1)

    return nc


def _get_nc():
    global _CACHED_NC
    if _CACHED_NC is None:
        _CACHED_NC = build_nc()
    return _CACHED_NC


def make_in_maps(input, target):
    inp = np.asarray(input, dtype=np.float32)
    tgt = np.asarray(target, dtype=np.float32)
    in_maps = []
    NPP = N // P   # 2048 pixels per partition
    for b in range(B):
        # class-outer per-tile blocks, concatenated: [C, NPP] per partition
        xc = inp[b].reshape(C, P, NPP).transpose(1, 0, 2)   # [P, C, NPP]
        yp = tgt[b].reshape(C, P, NPP).transpose(1, 2, 0)   # [P, NPP, C] pix-major
        xflat = np.empty((P, C * NPP), dtype=np.float32)
        for t in range(NT):
            k0, k1 = KOFF[t], KOFF[t] + KT[t]
            xflat[:, C * k0 : C * k1] = xc[:, :, k0:k1].reshape(P, C * KT[t])
        # interleave pixel pairs: pack q cols m = 2i+a = y[pixel 2q+a, class i]
        y2 = yp.reshape(P, NPP // 2, 2, C).transpose(0, 1, 3, 2)
        in_maps.append(
            {
                "x": xflat.astype(X_NP_DT),
                "y": np.ascontiguousarray(y2).astype(Y_NP_DT).reshape(P, C * NPP),
            }
        )
    return in_maps


def postprocess(outs):
    acc = np.stack([np.asarray(o, dtype=np.float64) for o in outs])  # [B, P, 2C]
    raw = 0
    for j in range(2):  # column tile lanes
        blk = acc[:, 64 * j : 64 * j + 2 * C, :].reshape(-1, C, 2, C, 2)
        raw = raw + blk[:, :, 0, :, 0] + blk[:, :, 1, :, 1]
    cm = raw / (raw.sum(axis=2, keepdims=True) + 1e-30)
    return cm.mean(axis=0).astype(np.float32)


def kernel(input, target):
    nc = _get_nc()
    in_maps = make_in_maps(input, target)
    res = run_bass_kernel_spmd(nc, in_maps, list(range(N_CORES)))
    return postprocess([r["out"] for r in res.results])
